# revision 1
# baseline (speedup 1.0000x reference)
"""Bass/Tile kernel builder for BSplineField3d (tricubic B-spline interpolation).

Algorithm (per NeuronCore, data-parallel over points):
  Phase 1 (build): from phi [128,128,128,3] build Cy [128x, 125yc, 128z, 3c*4k]
    where Cy[x,yc,z,c,k] = sum_m A[k,m] * phi[x, yc+m, z, c].
    The y-dimension B-spline is thus pre-contracted into per-cell polynomial
    coefficients in v (the fractional y coordinate), making each point's
    required data a CONTIGUOUS 48-float record per x-tap:
        rec(x, yc, z0) = Cy[x, yc, z0:z0+4, :, :]   (z-window x c x k)
    Built with PE matmuls against a constant banded matrix W[y, (k,yc)].
  Phase 2 (points): per chunk of 128x64 points:
    - compute cell indices + fractional coords on DVE
    - indirect DMA gather: 4 records per point (x-taps l=0..3), 192B each
    - combine with weights M[l,n,k] = wu_l * ww_n * v^k on DVE:
        T[c] = sum_{l,n,k} M * R[l, n, c, k]
"""

from contextlib import ExitStack

import sys as _sys
for _p in ("/opt/trn_rl_repo",):
    if _p not in _sys.path:
        _sys.path.append(_p)

import numpy as np

import concourse.bass as bass
import concourse.tile as tile
from concourse import mybir
from concourse._compat import with_exitstack

F32 = mybir.dt.float32
I32 = mybir.dt.int32

NX = 128          # grid points per dim
NCELL = 125       # valid cells per dim (ix in [0,124])
NC_ = 3           # components
ZC = NX * NC_     # 384 floats per (x,y) z-row in phi
KC = 12           # (c=3, k=4) floats per (x,yc,z) in Cy
ROW = NX * KC     # 1536 floats per (x,yc) in Cy
QROW = NX * 48    # 6144 floats per (xq,yc): [z, xs4, c3, k4]
CY_ELEMS = NX * NCELL * ROW  # 24,576,000 (98.3 MB fp32)

COLS = 1984       # points per partition (128*1984 = 253952 >= 250000)
P = 32            # points per partition per chunk
NCHUNK = COLS // P  # 31

# spacing: dx = 2/(nx-3) = 2/125 -> 1/dx = 62.5; u = (x+1)*62.5
INV_D = 62.5
XSTRIDE = NCELL * NX  # 16000: record-index stride for x (in 12-float units)


def bspline_poly_A():
    """A[k][m]: coefficient of v^k in the cubic B-spline weight of tap m."""
    return np.array(
        [
            [1 / 6, 4 / 6, 1 / 6, 0.0],
            [-3 / 6, 0.0, 3 / 6, 0.0],
            [3 / 6, -6 / 6, 3 / 6, 0.0],
            [-1 / 6, 3 / 6, -3 / 6, 1 / 6],
        ],
        dtype=np.float64,
    )


def build_W_const():
    """W[y, k*125+yc] = A[k, y-yc] for 0 <= y-yc <= 3 else 0. Shape [128, 500]."""
    A = bspline_poly_A()
    W = np.zeros((128, 4, 125), np.float32)
    for yc in range(NCELL):
        for m in range(4):
            for k in range(4):
                W[yc + m, k, yc] = A[k, m]
    return W.reshape(128, 500)


def _ap(t, offset, dims):
    """Build a raw AP on the same tensor as AP `t` with explicit [step, num] dims."""
    return bass.AP(tensor=t.tensor, offset=t.offset + offset, ap=[list(d) for d in dims])


@with_exitstack
def bspline_kernel(ctx: ExitStack, tc: tile.TileContext, outs, ins):
    """outs = [T_out [128, COLS, 3] f32]; ins = [xs, ys, zs [128, COLS] f32, phi [128,128,384] f32]."""
    nc = tc.nc
    xs, ys, zs, phi = ins
    t_out = outs[0]

    w_np = build_W_const()
    w_dram = nc.inline_tensor(w_np, name="w_const")

    dram = ctx.enter_context(tc.tile_pool(name="cydram", bufs=1, space="DRAM"))
    cy = dram.tile([NX // 4, NCELL, QROW], F32)

    # ---------------- Phase 1: build Cy ----------------
    with ExitStack() as p1:
        singles = p1.enter_context(tc.tile_pool(name="p1_singles", bufs=1))
        phis = p1.enter_context(tc.tile_pool(name="p1_phi", bufs=3))
        stages = p1.enter_context(tc.tile_pool(name="p1_stage", bufs=3))
        psums = p1.enter_context(tc.psum_pool(name="p1_psum", bufs=8))

        w_sb = singles.tile([128, 500], F32)
        nc.default_dma_engine.dma_start(out=w_sb[:], in_=w_dram.ap())

        for xq in range(NX // 4):
            stage = stages.tile([128, QROW], F32)  # [yc, z*48 + xs*12 + c*4 + k]
            for xsl in range(4):
                phi_x = phis.tile([128, ZC], F32, name=f"phi_{xsl}")
                nc.default_dma_engine.dma_start(out=phi_x[:], in_=phi[4 * xq + xsl])
                for k in range(4):
                    ps = psums.tile([NCELL, ZC], F32)
                    nc.tensor.matmul(
                        ps[:],
                        w_sb[:, k * NCELL:(k + 1) * NCELL],
                        phi_x[:],
                        start=True,
                        stop=True,
                    )
                    # psum [yc, (z,c)] -> stage[yc, z*48 + xs*12 + c*4 + k]
                    eng = nc.scalar if (k % 2 == 0) else nc.vector
                    src = _ap(ps[:], 0, [[ZC, NCELL], [3, NX], [1, NC_]])
                    dst = _ap(stage[:], xsl * KC + k,
                              [[QROW, NCELL], [48, NX], [4, NC_]])
                    if eng is nc.scalar:
                        eng.copy(out=dst, in_=src)
                    else:
                        eng.tensor_copy(out=dst, in_=src)
            nc.default_dma_engine.dma_start(
                out=cy[xq, :, :], in_=stage[:NCELL, :]
            )

    # ---------------- Phase 2: points ----------------
    with ExitStack() as p2:
        sing = p2.enter_context(tc.tile_pool(name="p2_singles", bufs=1))
        coords = p2.enter_context(tc.tile_pool(name="p2_coords", bufs=3))
        small = p2.enter_context(tc.tile_pool(name="p2_small", bufs=2))
        idxp = p2.enter_context(tc.tile_pool(name="p2_idx", bufs=3))
        recs = p2.enter_context(tc.tile_pool(name="p2_rec", bufs=2))
        prods = p2.enter_context(tc.tile_pool(name="p2_prod", bufs=2))
        touts = p2.enter_context(tc.tile_pool(name="p2_tout", bufs=2))

        # j-ramp constant: [128, 8] = 0..7 (x-slot index within gathered pair)
        jr8 = sing.tile([128, 8], F32)
        nc.gpsimd.iota(jr8[:], [[1, 8]], channel_multiplier=0,
                       allow_small_or_imprecise_dtypes=True)

        cy_flat = _ap(cy[:], 0, [[48, CY_ELEMS // 48], [1, 48]])

        for ch in range(NCHUNK):
            x_t = coords.tile([128, P], F32)
            y_t = coords.tile([128, P], F32)
            z_t = coords.tile([128, P], F32)
            nc.default_dma_engine.dma_start(out=x_t[:], in_=xs[:, ch * P:(ch + 1) * P])
            nc.default_dma_engine.dma_start(out=y_t[:], in_=ys[:, ch * P:(ch + 1) * P])
            nc.default_dma_engine.dma_start(out=z_t[:], in_=zs[:, ch * P:(ch + 1) * P])

            # --- cell indices + fractions (DVE) ---
            def exact_floor(src, out, sfx):
                # out = floor(src) for src >= 0, robust to cast rounding mode
                ci = small.tile([128, P], I32, name=f"ci_{sfx}")
                cf = small.tile([128, P], F32, name=f"cf_{sfx}")
                nc.vector.tensor_copy(out=ci[:], in_=src[:])
                nc.vector.tensor_copy(out=cf[:], in_=ci[:])
                nc.vector.tensor_tensor(out[:], cf[:], src[:], mybir.AluOpType.is_gt)
                nc.vector.tensor_sub(out[:], cf[:], out[:])

            def split_coord(src, sfx):
                u = small.tile([128, P], F32, name=f"u_{sfx}")
                fr = small.tile([128, P], F32, name=f"fr_{sfx}")
                ii = small.tile([128, P], F32, name=f"ii_{sfx}")
                nc.vector.tensor_scalar(u[:], src[:], 1.0, INV_D,
                                        mybir.AluOpType.add, mybir.AluOpType.mult)
                exact_floor(u, ii, sfx)
                nc.vector.tensor_sub(fr[:], u[:], ii[:])
                nc.vector.tensor_scalar(ii[:], ii[:], float(NCELL - 1), 0.0,
                                        mybir.AluOpType.min, mybir.AluOpType.max)
                return ii, fr

            ix_t, fu = split_coord(x_t, "x")
            iy_t, fv = split_coord(y_t, "y")
            iz_t, fw = split_coord(z_t, "z")

            # --- x-quad decomposition: xq = ix//4, s0 = ix%4, xq1 = min(xq+1,31)
            tq = small.tile([128, P], F32)
            xq_t = small.tile([128, P], F32)
            xq1_t = small.tile([128, P], F32)
            s0_t = small.tile([128, P], F32)
            nc.vector.tensor_scalar(tq[:], ix_t[:], 0.25, None, mybir.AluOpType.mult)
            exact_floor(tq, xq_t, "q")
            nc.vector.scalar_tensor_tensor(
                s0_t[:], xq_t[:], -4.0, ix_t[:],
                mybir.AluOpType.mult, mybir.AluOpType.add)
            nc.vector.tensor_scalar(xq1_t[:], xq_t[:], 1.0, 31.0,
                                    mybir.AluOpType.add, mybir.AluOpType.min)

            # --- record indices (48-float units): xqg*16000 + iy*128 + iz
            byz = small.tile([128, P], F32)
            nc.vector.scalar_tensor_tensor(
                byz[:], iy_t[:], float(NX), iz_t[:],
                mybir.AluOpType.mult, mybir.AluOpType.add)
            idx_f = idxp.tile([128, P, 2], F32)
            nc.vector.scalar_tensor_tensor(
                _ap(idx_f[:], 0, [[P * 2, 128], [2, P]]),
                xq_t[:], float(XSTRIDE), byz[:],
                mybir.AluOpType.mult, mybir.AluOpType.add)
            nc.vector.scalar_tensor_tensor(
                _ap(idx_f[:], 1, [[P * 2, 128], [2, P]]),
                xq1_t[:], float(XSTRIDE), byz[:],
                mybir.AluOpType.mult, mybir.AluOpType.add)
            idx_i = idxp.tile([128, P * 2], I32)
            nc.vector.tensor_copy(out=idx_i[:], in_=idx_f[:].rearrange("p a b -> p (a b)"))

            # --- gather: 2 records x 192 floats per point (vector-mode: one
            # descriptor per partition per instruction) ---
            rec = recs.tile([128, P * 2 * 192], F32)
            for t in range(P * 2):
                nc.gpsimd.indirect_dma_start(
                    out=_ap(rec[:], t * 192, [[P * 2 * 192, 128], [1, 192]]),
                    out_offset=None,
                    in_=cy_flat,
                    in_offset=bass.IndirectOffsetOnAxis(
                        ap=_ap(idx_i[:], t, [[P * 2, 128], [1, 1]]), axis=0),
                )

            # --- weights ---
            def tap_weights(fr, sfx):
                wt = small.tile([128, P, 4], F32, name=f"wt_{sfx}")
                t = small.tile([128, P], F32, name=f"t_{sfx}")
                t2 = small.tile([128, P], F32, name=f"t2_{sfx}")
                r2 = small.tile([128, P], F32, name=f"r2_{sfx}")
                r3 = small.tile([128, P], F32, name=f"r3_{sfx}")
                w0 = wt[:, :, 0]
                w1 = wt[:, :, 1]
                w2 = wt[:, :, 2]
                w3 = wt[:, :, 3]
                nc.vector.tensor_scalar(t[:], fr[:], -1.0, 1.0,
                                        mybir.AluOpType.mult, mybir.AluOpType.add)
                nc.vector.tensor_mul(t2[:], t[:], t[:])
                nc.vector.scalar_tensor_tensor(w0, t2[:], 1 / 6, t[:],
                                               mybir.AluOpType.mult, mybir.AluOpType.mult)
                nc.vector.tensor_mul(r2[:], fr[:], fr[:])
                nc.vector.tensor_mul(r3[:], r2[:], fr[:])
                nc.vector.tensor_scalar(w3, r3[:], 1 / 6, None, mybir.AluOpType.mult)
                nc.vector.scalar_tensor_tensor(w1, r3[:], 0.5, r2[:],
                                               mybir.AluOpType.mult, mybir.AluOpType.subtract)
                nc.vector.tensor_scalar(w1, w1, 2 / 3, None, mybir.AluOpType.add)
                nc.vector.tensor_add(w2, w0, w1)
                nc.vector.tensor_add(w2, w2, w3)
                nc.vector.tensor_scalar(w2, w2, -1.0, 1.0,
                                        mybir.AluOpType.mult, mybir.AluOpType.add)
                return wt

            wu = tap_weights(fu, "u")
            ww = tap_weights(fw, "w")

            vp = small.tile([128, P, 4], F32)
            nc.vector.memset(vp[:, :, 0], 1.0)
            nc.vector.tensor_copy(out=vp[:, :, 1], in_=fv[:])
            nc.vector.tensor_mul(vp[:, :, 2], fv[:], fv[:])
            nc.vector.tensor_mul(vp[:, :, 3], vp[:, :, 2], fv[:])

            # --- W8[pt, j] = wu[j - s0] for j-s0 in [0,4) else 0 ---
            d8 = small.tile([128, P, 8], F32)
            e8 = small.tile([128, P, 8], F32)
            w8 = small.tile([128, P, 8], F32)
            nc.vector.tensor_tensor(
                _ap(d8[:], 0, [[P * 8, 128], [8, P], [1, 8]]),
                _ap(jr8[:], 0, [[8, 128], [0, P], [1, 8]]),
                _ap(s0_t[:], 0, [[P, 128], [1, P], [0, 8]]),
                mybir.AluOpType.subtract)
            for l in range(4):
                tgt = w8 if l == 0 else e8
                nc.vector.tensor_scalar(e8[:], d8[:], float(l), None,
                                        mybir.AluOpType.is_equal)
                nc.vector.tensor_tensor(
                    _ap(tgt[:], 0, [[P * 8, 128], [8, P], [1, 8]]),
                    _ap(e8[:], 0, [[P * 8, 128], [8, P], [1, 8]]),
                    _ap(wu[:], l, [[P * 4, 128], [4, P], [0, 8]]),
                    mybir.AluOpType.mult)
                if l > 0:
                    nc.vector.tensor_add(w8[:], w8[:], e8[:])

            # --- contraction: T[c] = sum_{g,z,x,k} w8[gx]*ww[z]*v^k * R ---
            # rec per point: [g2][z4][x4][c3][k4] (gzxc=96, k innermost)
            # 1) contract k with vp (in-place into rec), reduce -> s1 [gzxc]
            nc.vector.tensor_tensor(
                _ap(rec[:], 0, [[P * 384, 128], [384, P], [4, 96], [1, 4]]),
                _ap(rec[:], 0, [[P * 384, 128], [384, P], [4, 96], [1, 4]]),
                _ap(vp[:], 0, [[P * 4, 128], [4, P], [0, 96], [1, 4]]),
                mybir.AluOpType.mult)
            s1 = prods.tile([128, P * 96], F32)
            nc.vector.tensor_reduce(
                out=s1[:],
                in_=_ap(rec[:], 0, [[P * 384, 128], [4, P * 96], [1, 4]]),
                axis=mybir.AxisListType.X,
                op=mybir.AluOpType.add)
            # 2) contract z with ww; write transposed so z is innermost
            t1 = prods.tile([128, P * 96], F32)
            for g in range(2):
                nc.vector.tensor_tensor(
                    _ap(t1[:], g * 48, [[P * 96, 128], [96, P], [1, 4], [4, 12]]),
                    _ap(s1[:], g * 48, [[P * 96, 128], [96, P], [12, 4], [1, 12]]),
                    _ap(ww[:], 0, [[P * 4, 128], [4, P], [1, 4], [0, 12]]),
                    mybir.AluOpType.mult)
            s2 = touts.tile([128, P * 24], F32)
            nc.vector.tensor_reduce(
                out=s2[:],
                in_=_ap(t1[:], 0, [[P * 96, 128], [4, P * 24], [1, 4]]),
                axis=mybir.AxisListType.X,
                op=mybir.AluOpType.add)
            # 3) contract (g,x) with w8; write transposed so gx is innermost
            t2 = touts.tile([128, P * 24], F32)
            nc.vector.tensor_tensor(
                _ap(t2[:], 0, [[P * 24, 128], [24, P], [1, 8], [8, 3]]),
                _ap(s2[:], 0, [[P * 24, 128], [24, P], [3, 8], [1, 3]]),
                _ap(w8[:], 0, [[P * 8, 128], [8, P], [1, 8], [0, 3]]),
                mybir.AluOpType.mult)
            t_c = touts.tile([128, P * 3], F32)
            nc.vector.tensor_reduce(
                out=t_c[:],
                in_=_ap(t2[:], 0, [[P * 24, 128], [8, P * 3], [1, 8]]),
                axis=mybir.AxisListType.X,
                op=mybir.AluOpType.add)

            nc.default_dma_engine.dma_start(
                out=t_out[:, ch * P:(ch + 1) * P, :],
                in_=t_c[:].rearrange("p (a b) -> p a b", b=3))


# ======================================================================
# Self-contained entry point: kernel(**inputs) -> np.ndarray
# ======================================================================

N_POINTS = 2_000_000
N_CORES = 8
PTS_PER_CORE = N_POINTS // N_CORES      # 250000
PAD_PER_CORE = 128 * COLS               # 253952

_CACHE = {}


def _build_nc():
    import concourse.bacc as bacc

    nc = bacc.Bacc(
        "TRN2",
        target_bir_lowering=False,
        debug=False,
        num_devices=N_CORES,
    )
    xs = nc.dram_tensor("xs", [128, COLS], F32, kind="ExternalInput").ap()
    ys = nc.dram_tensor("ys", [128, COLS], F32, kind="ExternalInput").ap()
    zs = nc.dram_tensor("zs", [128, COLS], F32, kind="ExternalInput").ap()
    phi = nc.dram_tensor("phi", [128, 128, ZC], F32, kind="ExternalInput").ap()
    t_out = nc.dram_tensor("t_out", [128, COLS, NC_], F32, kind="ExternalOutput").ap()

    with tile.TileContext(nc) as tc:
        bspline_kernel(tc, [t_out], [xs, ys, zs, phi])
    nc.compile()
    return nc


def get_nc():
    if "nc" not in _CACHE:
        _CACHE["nc"] = _build_nc()
    return _CACHE["nc"]


def _shard(arr):
    """[N_POINTS] -> list of 8 [128, COLS] arrays (padded with zeros)."""
    out = []
    for c in range(N_CORES):
        s = arr[c * PTS_PER_CORE:(c + 1) * PTS_PER_CORE]
        p = np.zeros(PAD_PER_CORE, dtype=np.float32)
        p[:PTS_PER_CORE] = s
        out.append(p.reshape(128, COLS))
    return out


def run_on_cores(x, y, z, phi_x, trace=False, **kw):
    from concourse.bass_utils import run_bass_kernel_spmd

    nc = get_nc()
    xsh, ysh, zsh = _shard(x), _shard(y), _shard(z)
    phi_r = np.ascontiguousarray(phi_x.reshape(128, 128, ZC))
    in_maps = [
        {"xs": xsh[c], "ys": ysh[c], "zs": zsh[c], "phi": phi_r}
        for c in range(N_CORES)
    ]
    res = run_bass_kernel_spmd(
        nc, in_maps, core_ids=list(range(N_CORES)), trace=trace, **kw
    )
    outs = []
    for c in range(N_CORES):
        t = res.results[c]["t_out"].reshape(PAD_PER_CORE, NC_)
        outs.append(t[:PTS_PER_CORE])
    full = np.concatenate(outs, axis=0).astype(np.float32)
    return full, res


def kernel(x, y, z, phi_x):
    full, _ = run_on_cores(
        np.asarray(x, dtype=np.float32),
        np.asarray(y, dtype=np.float32),
        np.asarray(z, dtype=np.float32),
        np.asarray(phi_x, dtype=np.float32),
    )
    return full



# revision 8
# speedup vs baseline: 1.9531x; 1.9531x over previous
"""Bass/Tile kernel for BSplineField3d (tricubic B-spline interpolation).

v2 design (cost-model-driven):
  Table: Cy8[xq=32, yc=125, z=128, xs=8, c=3, ky=4] in bf16 (98 MB DRAM).
    Cy8[xq,yc,z,xs,c,ky] = sum_m A[ky,m] * phi[min(4*xq+xs,127), yc+m, z, c]
    The y-dimension is pre-contracted into a degree-3 polynomial in v
    (coefficient index ky); the 8-wide x-slot span (xs) makes each point's
    full 64-tap data ONE contiguous 768-byte record:
        rec(xq, yc, z0) = Cy8[xq, yc, z0:z0+4, :, :, :]   (384 bf16)
    built with PE matmuls (bf16) against a banded B-spline matrix.
  Phase 2: per chunk of 128xP points:
    - cell indices + fractional coords (DVE/Act)
    - ONE indirect-DMA record per point (1984 gather instructions total,
      each 128 records of 768B, issued on gpsimd)
    - combine on DVE in bf16 (2x packed mode):
        mult by W16[z,ky] = ww[z]*v^ky, tree-reduce z, tree-reduce ky,
        mult by W8c[xs,c] = w8[xs] (masked wu), tree-reduce xs.
"""

from contextlib import ExitStack

import sys as _sys
for _p in ("/opt/trn_rl_repo",):
    if _p not in _sys.path:
        _sys.path.append(_p)

import numpy as np

import concourse.bass as bass
import concourse.tile as tile
from concourse import mybir
from concourse._compat import with_exitstack

F32 = mybir.dt.float32
BF16 = mybir.dt.bfloat16
I32 = mybir.dt.int32

NX = 128          # grid points per dim
NCELL = 125       # valid cells per dim (ix in [0,124])
NC_ = 3           # components
ZC = NX * NC_     # 384 floats per (y,x) z-row in transposed phi
UNIT = 96         # bf16 elems per (xq,yc,z): [xs8, c3, ky4]
RECE = 4 * UNIT   # 384 elems per record (z-window of 4 units)
NXQ = 32
TAB_ELEMS = NXQ * NCELL * NX * UNIT  # 49,152,000

COLS = 1984       # points per partition (128*1984 = 253952 >= 250000)
P = 64            # points per partition per chunk
NCHUNK = COLS // P  # 31

INV_D = 62.5      # 1/dx, dx = 2/125


def bspline_poly_A():
    """A[k][m]: coefficient of v^k in the cubic B-spline weight of tap m."""
    return np.array(
        [
            [1 / 6, 4 / 6, 1 / 6, 0.0],
            [-3 / 6, 0.0, 3 / 6, 0.0],
            [3 / 6, -6 / 6, 3 / 6, 0.0],
            [-1 / 6, 3 / 6, -3 / 6, 1 / 6],
        ],
        dtype=np.float64,
    )


def build_W_const():
    """W[y, ky*125+yc] = A[ky, y-yc] for 0 <= y-yc <= 3 else 0; bf16 [128, 500]."""
    import ml_dtypes
    A = bspline_poly_A()
    W = np.zeros((128, 4, 125), np.float32)
    for yc in range(NCELL):
        for m in range(4):
            for k in range(4):
                W[yc + m, k, yc] = A[k, m]
    return W.reshape(128, 500).astype(ml_dtypes.bfloat16)


def _ap(t, offset, dims):
    """Raw AP on the same tensor as AP `t` with explicit [step, num] dims."""
    return bass.AP(tensor=t.tensor, offset=t.offset + offset, ap=[list(d) for d in dims])


@with_exitstack
def bspline_kernel(ctx: ExitStack, tc: tile.TileContext, outs, ins):
    """outs = [T_out [128, COLS, 3] f32]
    ins  = [xs, ys, zs [128, COLS] f32, phi_t [128, 49152] bf16 (y-major)]"""
    nc = tc.nc
    xs, ys, zs, phi = ins
    t_out = outs[0]

    w_dram = nc.inline_tensor(build_W_const(), name="w_const")

    dram = ctx.enter_context(tc.tile_pool(name="cydram", bufs=1, space="DRAM"))
    cy = dram.tile([NXQ, NCELL, NX * UNIT], BF16)

    # ---------------- Phase 1: build Cy8 ----------------
    with ExitStack() as p1:
        singles = p1.enter_context(tc.tile_pool(name="p1_singles", bufs=1))
        stages = p1.enter_context(tc.tile_pool(name="p1_stage", bufs=3))
        psums = p1.enter_context(tc.psum_pool(name="p1_psum", bufs=8))

        w_sb = singles.tile([128, 500], BF16)
        nc.sync.dma_start(out=w_sb[:], in_=w_dram.ap())
        phi_sb = singles.tile([128, 128 * ZC], BF16)
        # 4 loads so they pipeline with the first matmuls
        for q in range(4):
            nc.sync.dma_start(
                out=phi_sb[:, q * 32 * ZC:(q + 1) * 32 * ZC],
                in_=_ap(phi, q * 32 * ZC, [[128 * ZC, 128], [1, 32 * ZC]]),
            )

        wr_engines = [nc.sync, nc.gpsimd, nc.sync, nc.gpsimd]
        for xq in range(NXQ):
            stage = stages.tile([NCELL, NX * UNIT], BF16)
            for xsl in range(8):
                x = min(4 * xq + xsl, 127)
                for ky in range(4):
                    ps = psums.tile([NCELL, ZC], F32)
                    nc.tensor.matmul(
                        ps[:],
                        w_sb[:, ky * NCELL:(ky + 1) * NCELL],
                        phi_sb[:, x * ZC:(x + 1) * ZC],
                        start=True,
                        stop=True,
                    )
                    # psum [yc, (z,c)] f32 -> stage[yc, z*96 + xsl*12 + c*4 + ky] bf16
                    src = _ap(ps[:], 0, [[ZC, NCELL], [3, NX], [1, NC_]])
                    dst = _ap(stage[:], xsl * 12 + ky,
                              [[NX * UNIT, NCELL], [UNIT, NX], [4, NC_]])
                    if (xsl + ky) % 2 == 0:
                        nc.scalar.copy(out=dst, in_=src)
                    else:
                        nc.vector.tensor_copy(out=dst, in_=src)
            wr_engines[xq % len(wr_engines)].dma_start(
                out=cy[xq, :, :], in_=stage[:NCELL, :])

    # ---------------- Phase 2: points ----------------
    cy_flat = _ap(cy[:], 0, [[TAB_ELEMS, 1], [1, TAB_ELEMS]])

    with ExitStack() as p2:
        sing = p2.enter_context(tc.tile_pool(name="p2_singles", bufs=1))
        coords = p2.enter_context(tc.tile_pool(name="p2_coords", bufs=3))
        small = p2.enter_context(tc.tile_pool(name="p2_small", bufs=2))
        idxp = p2.enter_context(tc.tile_pool(name="p2_idx", bufs=3))
        recs = p2.enter_context(tc.tile_pool(name="p2_rec", bufs=2))
        prods = p2.enter_context(tc.tile_pool(name="p2_prod", bufs=1))
        touts = p2.enter_context(tc.tile_pool(name="p2_tout", bufs=2))

        # j-ramp constant: [128, 8] = 0..7 (x-slot index within record)
        jr8 = sing.tile([128, 8], F32)
        nc.gpsimd.iota(jr8[:], [[1, 8]], channel_multiplier=0,
                       allow_small_or_imprecise_dtypes=True)

        for ch in range(NCHUNK):
            x_t = coords.tile([128, P], F32)
            y_t = coords.tile([128, P], F32)
            z_t = coords.tile([128, P], F32)
            nc.sync.dma_start(out=x_t[:], in_=xs[:, ch * P:(ch + 1) * P])
            nc.sync.dma_start(out=y_t[:], in_=ys[:, ch * P:(ch + 1) * P])
            nc.sync.dma_start(out=z_t[:], in_=zs[:, ch * P:(ch + 1) * P])

            # --- cell indices + fractions ---
            def exact_floor(src, out, sfx):
                # out = floor(src) for src >= 0, robust to cast rounding mode
                ci = small.tile([128, P], I32, name=f"ci_{sfx}")
                cf = small.tile([128, P], F32, name=f"cf_{sfx}")
                nc.vector.tensor_copy(out=ci[:], in_=src[:])
                nc.vector.tensor_copy(out=cf[:], in_=ci[:])
                nc.vector.tensor_tensor(out[:], cf[:], src[:], mybir.AluOpType.is_gt)
                nc.vector.tensor_sub(out[:], cf[:], out[:])

            def split_coord(src, sfx):
                u = small.tile([128, P], F32, name=f"u_{sfx}")
                fr = small.tile([128, P], F32, name=f"fr_{sfx}")
                ii = small.tile([128, P], F32, name=f"ii_{sfx}")
                nc.vector.tensor_scalar(u[:], src[:], 1.0, INV_D,
                                        mybir.AluOpType.add, mybir.AluOpType.mult)
                exact_floor(u, ii, sfx)
                nc.vector.tensor_scalar(ii[:], ii[:], float(NCELL - 1), 0.0,
                                        mybir.AluOpType.min, mybir.AluOpType.max)
                nc.vector.tensor_sub(fr[:], u[:], ii[:])
                return ii, fr

            ix_t, fu = split_coord(x_t, "x")
            iy_t, fv = split_coord(y_t, "y")
            iz_t, fw = split_coord(z_t, "z")

            # --- x-quad decomposition: xq = ix//4, s0 = ix%4 ---
            tq = small.tile([128, P], F32)
            xq_t = small.tile([128, P], F32)
            s0_t = small.tile([128, P], F32)
            nc.vector.tensor_scalar(tq[:], ix_t[:], 0.25, None, mybir.AluOpType.mult)
            exact_floor(tq, xq_t, "q")
            nc.vector.scalar_tensor_tensor(
                s0_t[:], xq_t[:], -4.0, ix_t[:],
                mybir.AluOpType.mult, mybir.AluOpType.add)

            # --- record index (elem units): ((xq*125+yc)*128+z0)*96
            #     = 32 * (3 * (xq*16000 + yc*128 + z0))
            byz = small.tile([128, P], F32)
            nc.vector.scalar_tensor_tensor(
                byz[:], iy_t[:], float(NX), iz_t[:],
                mybir.AluOpType.mult, mybir.AluOpType.add)
            idx_f = small.tile([128, P], F32)
            nc.vector.scalar_tensor_tensor(
                idx_f[:], xq_t[:], 16000.0, byz[:],
                mybir.AluOpType.mult, mybir.AluOpType.add)
            nc.vector.tensor_scalar(idx_f[:], idx_f[:], 3.0, None,
                                    mybir.AluOpType.mult)
            idx_i = idxp.tile([128, P], I32)
            nc.vector.tensor_copy(out=idx_i[:], in_=idx_f[:])
            # *32 via 5 int32 doublings (int immediates on tensor_scalar are
            # not reliably supported)
            for _ in range(5):
                nc.vector.tensor_add(idx_i[:], idx_i[:], idx_i[:])

            # --- tap weights ---
            def tap_weights(fr, sfx):
                wt = small.tile([128, P, 4], F32, name=f"wt_{sfx}")
                t = small.tile([128, P], F32, name=f"t_{sfx}")
                t2 = small.tile([128, P], F32, name=f"t2_{sfx}")
                r2 = small.tile([128, P], F32, name=f"r2_{sfx}")
                r3 = small.tile([128, P], F32, name=f"r3_{sfx}")
                w0 = wt[:, :, 0]
                w1 = wt[:, :, 1]
                w2 = wt[:, :, 2]
                w3 = wt[:, :, 3]
                nc.vector.tensor_scalar(t[:], fr[:], -1.0, 1.0,
                                        mybir.AluOpType.mult, mybir.AluOpType.add)
                nc.vector.tensor_mul(t2[:], t[:], t[:])
                nc.vector.scalar_tensor_tensor(w0, t2[:], 1 / 6, t[:],
                                               mybir.AluOpType.mult, mybir.AluOpType.mult)
                nc.vector.tensor_mul(r2[:], fr[:], fr[:])
                nc.vector.tensor_mul(r3[:], r2[:], fr[:])
                nc.vector.tensor_scalar(w3, r3[:], 1 / 6, None, mybir.AluOpType.mult)
                nc.vector.scalar_tensor_tensor(w1, r3[:], 0.5, r2[:],
                                               mybir.AluOpType.mult, mybir.AluOpType.subtract)
                nc.vector.tensor_scalar(w1, w1, 2 / 3, None, mybir.AluOpType.add)
                nc.vector.tensor_add(w2, w0, w1)
                nc.vector.tensor_add(w2, w2, w3)
                nc.vector.tensor_scalar(w2, w2, -1.0, 1.0,
                                        mybir.AluOpType.mult, mybir.AluOpType.add)
                return wt

            wu = tap_weights(fu, "u")
            ww = tap_weights(fw, "w")

            vp = small.tile([128, P, 4], F32)
            nc.vector.memset(vp[:, :, 0], 1.0)
            nc.vector.tensor_copy(out=vp[:, :, 1], in_=fv[:])
            nc.vector.tensor_mul(vp[:, :, 2], fv[:], fv[:])
            nc.vector.tensor_mul(vp[:, :, 3], vp[:, :, 2], fv[:])

            # --- W16[pt, z4, ky4] = ww[z] * v^ky  (bf16) ---
            w16 = small.tile([128, P, 16], BF16)
            nc.vector.tensor_tensor(
                _ap(w16[:], 0, [[P * 16, 128], [16, P], [4, 4], [1, 4]]),
                _ap(ww[:], 0, [[P * 4, 128], [4, P], [1, 4], [0, 4]]),
                _ap(vp[:], 0, [[P * 4, 128], [4, P], [0, 4], [1, 4]]),
                mybir.AluOpType.mult)

            # --- w8[pt, j] = wu[j - s0] for j-s0 in [0,4) else 0 ---
            d8 = small.tile([128, P, 8], F32)
            e8 = small.tile([128, P, 8], F32)
            w8 = small.tile([128, P, 8], F32)
            nc.vector.tensor_tensor(
                _ap(d8[:], 0, [[P * 8, 128], [8, P], [1, 8]]),
                _ap(jr8[:], 0, [[8, 128], [0, P], [1, 8]]),
                _ap(s0_t[:], 0, [[P, 128], [1, P], [0, 8]]),
                mybir.AluOpType.subtract)
            for l in range(4):
                tgt = w8 if l == 0 else e8
                nc.vector.tensor_scalar(e8[:], d8[:], float(l), None,
                                        mybir.AluOpType.is_equal)
                nc.vector.tensor_tensor(
                    _ap(tgt[:], 0, [[P * 8, 128], [8, P], [1, 8]]),
                    _ap(e8[:], 0, [[P * 8, 128], [8, P], [1, 8]]),
                    _ap(wu[:], l, [[P * 4, 128], [4, P], [0, 8]]),
                    mybir.AluOpType.mult)
                if l > 0:
                    nc.vector.tensor_add(w8[:], w8[:], e8[:])

            # --- W8c[pt, xs8, c3] = w8[xs] replicated over c (bf16, Act) ---
            w8c = small.tile([128, P, 24], BF16)
            nc.scalar.copy(
                out=_ap(w8c[:], 0, [[P * 24, 128], [24, P], [3, 8], [1, 3]]),
                in_=_ap(w8[:], 0, [[P * 8, 128], [8, P], [1, 8], [0, 3]]))

            # --- gather: one 768B record per point ---
            rec = recs.tile([128, P * RECE], BF16)
            for t in range(P):
                nc.gpsimd.indirect_dma_start(
                    out=_ap(rec[:], t * RECE, [[P * RECE, 128], [1, RECE]]),
                    out_offset=None,
                    in_=cy_flat,
                    in_offset=bass.IndirectOffsetOnAxis(
                        ap=_ap(idx_i[:], t, [[P, 128], [1, 1]]), axis=1),
                )

            # --- combine ---
            # rec[pt, z4, xs8, c3, ky4]; iteration ((pt,z) merged, xsc24, ky).
            # (pt,z) merge is exact: rec stride 96 over P*4, w16 stride 4 over
            # P*4 (16 = 4*4).
            # 1) multiply by W16[z,ky] (bcast xs,c) -- bf16 2x
            nc.vector.tensor_tensor(
                _ap(rec[:], 0, [[P * RECE, 128], [96, P * 4], [4, 24], [1, 4]]),
                _ap(rec[:], 0, [[P * RECE, 128], [96, P * 4], [4, 24], [1, 4]]),
                _ap(w16[:], 0, [[P * 16, 128], [4, P * 4], [0, 24], [1, 4]]),
                mybir.AluOpType.mult)
            # 2) tree-reduce z (outer dim; fully packed)
            s192 = prods.tile([128, P * 192], BF16)
            nc.vector.tensor_tensor(
                _ap(s192[:], 0, [[P * 192, 128], [192, P], [96, 2], [1, 96]]),
                _ap(rec[:], 0, [[P * RECE, 128], [RECE, P], [96, 2], [1, 96]]),
                _ap(rec[:], 192, [[P * RECE, 128], [RECE, P], [96, 2], [1, 96]]),
                mybir.AluOpType.add)
            s96 = prods.tile([128, P * 96], BF16)
            nc.vector.tensor_tensor(
                _ap(s96[:], 0, [[P * 96, 128], [96, P], [1, 96]]),
                _ap(s192[:], 0, [[P * 192, 128], [192, P], [1, 96]]),
                _ap(s192[:], 96, [[P * 192, 128], [192, P], [1, 96]]),
                mybir.AluOpType.add)
            # 3) tree-reduce ky: s96[pt, xs8, c3, ky4] -> s24[pt, xs8, c3]
            s48 = prods.tile([128, P * 48], BF16)
            nc.vector.tensor_tensor(
                _ap(s48[:], 0, [[P * 48, 128], [48, P], [2, 24], [1, 2]]),
                _ap(s96[:], 0, [[P * 96, 128], [96, P], [4, 24], [1, 2]]),
                _ap(s96[:], 2, [[P * 96, 128], [96, P], [4, 24], [1, 2]]),
                mybir.AluOpType.add)
            s24 = prods.tile([128, P * 24], BF16)
            nc.vector.tensor_tensor(
                _ap(s24[:], 0, [[P * 24, 128], [24, P], [1, 24]]),
                _ap(s48[:], 0, [[P * 48, 128], [48, P], [2, 24]]),
                _ap(s48[:], 1, [[P * 48, 128], [48, P], [2, 24]]),
                mybir.AluOpType.add)
            # 4) multiply by W8c[xs,c] -- bf16 2x
            nc.vector.tensor_tensor(
                _ap(s24[:], 0, [[P * 24, 128], [1, P * 24]]),
                _ap(s24[:], 0, [[P * 24, 128], [1, P * 24]]),
                _ap(w8c[:], 0, [[P * 24, 128], [1, P * 24]]),
                mybir.AluOpType.mult)
            # 5) tree-reduce xs: [xs8, c3] -> [c3]
            s12 = touts.tile([128, P * 12], BF16)
            nc.vector.tensor_tensor(
                _ap(s12[:], 0, [[P * 12, 128], [12, P], [1, 12]]),
                _ap(s24[:], 0, [[P * 24, 128], [24, P], [1, 12]]),
                _ap(s24[:], 12, [[P * 24, 128], [24, P], [1, 12]]),
                mybir.AluOpType.add)
            s6 = touts.tile([128, P * 6], BF16)
            nc.vector.tensor_tensor(
                _ap(s6[:], 0, [[P * 6, 128], [6, P], [1, 6]]),
                _ap(s12[:], 0, [[P * 12, 128], [12, P], [1, 6]]),
                _ap(s12[:], 6, [[P * 12, 128], [12, P], [1, 6]]),
                mybir.AluOpType.add)
            t_c = touts.tile([128, P * 3], F32)
            nc.vector.tensor_tensor(
                _ap(t_c[:], 0, [[P * 3, 128], [3, P], [1, 3]]),
                _ap(s6[:], 0, [[P * 6, 128], [6, P], [1, 3]]),
                _ap(s6[:], 3, [[P * 6, 128], [6, P], [1, 3]]),
                mybir.AluOpType.add)

            nc.sync.dma_start(
                out=t_out[:, ch * P:(ch + 1) * P, :],
                in_=t_c[:].rearrange("p (a b) -> p a b", b=3))


# ======================================================================
# Self-contained entry point: kernel(**inputs) -> np.ndarray
# ======================================================================

N_POINTS = 2_000_000
N_CORES = 8
PTS_PER_CORE = N_POINTS // N_CORES      # 250000
PAD_PER_CORE = 128 * COLS               # 253952

_CACHE = {}


def _build_nc(trace_sim=False, compile_=True):
    import concourse.bacc as bacc

    nc = bacc.Bacc(
        "TRN2",
        target_bir_lowering=False,
        debug=False,
        num_devices=N_CORES,
    )
    xs = nc.dram_tensor("xs", [128, COLS], F32, kind="ExternalInput").ap()
    ys = nc.dram_tensor("ys", [128, COLS], F32, kind="ExternalInput").ap()
    zs = nc.dram_tensor("zs", [128, COLS], F32, kind="ExternalInput").ap()
    phi = nc.dram_tensor("phi", [128, 128 * ZC], BF16, kind="ExternalInput").ap()
    t_out = nc.dram_tensor("t_out", [128, COLS, NC_], F32, kind="ExternalOutput").ap()

    with tile.TileContext(nc, trace_sim=trace_sim) as tc:
        bspline_kernel(tc, [t_out], [xs, ys, zs, phi])
    if compile_:
        nc.compile()
    return nc


def get_nc():
    if "nc" not in _CACHE:
        _CACHE["nc"] = _build_nc()
    return _CACHE["nc"]


def _shard(arr):
    """[N_POINTS] -> list of 8 [128, COLS] arrays (padded with zeros)."""
    out = []
    for c in range(N_CORES):
        s = arr[c * PTS_PER_CORE:(c + 1) * PTS_PER_CORE]
        p = np.zeros(PAD_PER_CORE, dtype=np.float32)
        p[:PTS_PER_CORE] = s
        out.append(p.reshape(128, COLS))
    return out


def _prep_phi(phi_x):
    """[128,128,128,3] f32 x-major -> [y, x*(z*c)] bf16 as uint16 view."""
    import ml_dtypes
    pt = np.ascontiguousarray(phi_x.transpose(1, 0, 2, 3)).reshape(128, 128 * ZC)
    return pt.astype(ml_dtypes.bfloat16).view(np.uint16)


def run_on_cores(x, y, z, phi_x, trace=False, **kw):
    from concourse.bass_utils import run_bass_kernel_spmd

    nc = get_nc()
    xsh, ysh, zsh = _shard(x), _shard(y), _shard(z)
    phi_r = _prep_phi(phi_x)
    in_maps = [
        {"xs": xsh[c], "ys": ysh[c], "zs": zsh[c], "phi": phi_r}
        for c in range(N_CORES)
    ]
    res = run_bass_kernel_spmd(
        nc, in_maps, core_ids=list(range(N_CORES)), trace=trace, **kw
    )
    outs = []
    for c in range(N_CORES):
        t = res.results[c]["t_out"].reshape(PAD_PER_CORE, NC_)
        outs.append(t[:PTS_PER_CORE])
    full = np.concatenate(outs, axis=0).astype(np.float32)
    return full, res


def kernel(x, y, z, phi_x):
    full, _ = run_on_cores(
        np.asarray(x, dtype=np.float32),
        np.asarray(y, dtype=np.float32),
        np.asarray(z, dtype=np.float32),
        np.asarray(phi_x, dtype=np.float32),
    )
    return full
